# revision 26
# baseline (speedup 1.0000x reference)
"""Distributed causal self-attention on 8 TRN2 NeuronCores.

Strategy (tensor parallel on heads + per-batch staged AllToAll):
  - Each core owns 2 of the 16 heads: qkv projection for its heads (full
    batch/seq), causal attention in a transposed-score layout (scores
    [key, query] so softmax denominators come free from an appended
    ones-row on V; no row-max subtraction needed at these magnitudes).
  - Unnormalized y plus the per-(head,token) softmax denominator are
    resharded head-split -> token-split with one AllToAll per batch, so
    each collective fires as soon as its batch finishes and overlaps the
    next batch's compute. Normalization happens on the consumer side.
  - Each core computes the output projection for its token chunks.
  - Host gather is pure concatenation/reordering along tokens.

All TensorEngine-facing tensors keep the contraction dim on partitions
and use bf16 with full 128-row stationaries (q/k zero-padded per head)
so every matmul gets the fast-weight-load path. PSUM accumulation stays
fp32.
"""

import os
import sys

sys.path.insert(0, "/opt/trn_rl_repo")

import ml_dtypes
import numpy as np

import concourse.bass as bass
import concourse.mybir as mybir
import concourse.tile as tile
from concourse import bacc
from concourse.bass_utils import run_bass_kernel_spmd
from concourse.masks import make_identity


def _install_profile_hook():
    """The RL container's antenv stub lacks axon_hooks, so bass_utils can't
    reach the NTFF profiler. Recreate the tiny set/get module and wire it to
    trn_boot's ctypes hook against libaxon_pjrt.so."""
    import types

    if "antenv.axon_hooks" in sys.modules:
        return
    try:
        import antenv
        from trn_agent_boot.trn_boot import _ntff_profile_via_ctypes

        mod = types.ModuleType("antenv.axon_hooks")
        mod._hook = None

        def set_axon_ntff_profile_hook(h):
            mod._hook = h

        def get_axon_ntff_profile_hook():
            return mod._hook

        mod.set_axon_ntff_profile_hook = set_axon_ntff_profile_hook
        mod.get_axon_ntff_profile_hook = get_axon_ntff_profile_hook
        sys.modules["antenv.axon_hooks"] = mod
        antenv.axon_hooks = mod
        hook = _ntff_profile_via_ctypes("/opt/axon/libaxon_pjrt.so")
        if hook is not None:
            mod._hook = hook
    except Exception as e:  # profiling is best-effort; execution must work
        print(f"profile hook install failed: {e}", file=sys.stderr)


B, T, D, H, DH = 4, 2048, 1024, 16, 64
BT = B * T              # 8192 tokens
N_CORES = 8
HL = H // N_CORES       # 2 heads per core
FL = HL * DH            # 128 local features
TSLICE = BT // N_CORES  # 1024 output tokens per core
CH = TSLICE // B        # 256-token ownership chunk per (rank, batch)
SCALE = DH ** -0.5
F32 = mybir.dt.float32
BF16 = mybir.dt.bfloat16

IB = 512       # query block (free dim of transposed score matmuls)
NJ = T // 128  # 16 key tiles per (b, h)
AROW = FL + HL  # 130 rows per a2a block: 128 y rows + 2 den rows


def _build():
    nc = bacc.Bacc("TRN2", target_bir_lowering=False, debug=False,
                   num_devices=N_CORES)

    xT = nc.dram_tensor("xT", [D, BT], BF16, kind="ExternalInput")
    wqkvT = nc.dram_tensor("wqkvT", [D, 3 * FL], BF16, kind="ExternalInput")
    woutT = nc.dram_tensor("woutT", [D, D], BF16, kind="ExternalInput")
    selc = nc.dram_tensor("selc", [2 * N_CORES, 8, 128], BF16,
                          kind="ExternalInput")
    out = nc.dram_tensor("out", [TSLICE, D], F32, kind="ExternalOutput")

    xT_r = xT[:].rearrange("(o p) t -> p o t", p=128)        # [128, 8, BT]
    wqkvT_r = wqkvT[:].rearrange("(o p) f -> p o f", p=128)  # [128, 8, 384]
    woutT_r = woutT[:].rearrange("(o p) d -> p o d", p=128)  # [128, 8, 1024]

    with tile.TileContext(nc) as tc:
        from contextlib import ExitStack

        with ExitStack() as ctx:
            const = ctx.enter_context(tc.tile_pool(name="const", bufs=1))
            wpool = ctx.enter_context(tc.tile_pool(name="wpool", bufs=1))
            xpool = ctx.enter_context(tc.tile_pool(name="xpool", bufs=3))
            qkvpool = ctx.enter_context(tc.tile_pool(name="qkvpool", bufs=3))
            vpool = ctx.enter_context(tc.tile_pool(name="vpool", bufs=6))
            ppool = ctx.enter_context(tc.tile_pool(name="ppool", bufs=24))
            ydpool = ctx.enter_context(tc.tile_pool(name="ydpool", bufs=4))
            opool = ctx.enter_context(tc.tile_pool(name="opool", bufs=2))
            psA = ctx.enter_context(
                tc.tile_pool(name="psA", bufs=4, space="PSUM"))
            psY = ctx.enter_context(
                tc.tile_pool(name="psY", bufs=2, space="PSUM"))
            psT = ctx.enter_context(
                tc.tile_pool(name="psT", bufs=1, space="PSUM"))
            psN = ctx.enter_context(
                tc.tile_pool(name="psN", bufs=1, space="PSUM"))
            dram = ctx.enter_context(
                tc.tile_pool(name="dram", bufs=1, space="DRAM"))

            # ---- constants ----
            identity = const.tile([128, 128], BF16, tag="identity")
            make_identity(nc, identity[:])
            # masks[oi][p, f] = 1.0 where f - p - oi*128 >= 0 else 0
            # (keep key j0+p for query i0+f iff j <= i; offset oi*128 = j0-i0)
            masks = []
            for oi in range(4):
                m = const.tile([128, IB], BF16, tag=f"mask{oi}")
                nc.gpsimd.memset(m[:], 1.0)
                nc.gpsimd.affine_select(
                    out=m[:], in_=m[:],
                    compare_op=mybir.AluOpType.is_ge,
                    fill=0.0, base=-(oi * 128),
                    pattern=[[1, IB]], channel_multiplier=-1,
                )
                masks.append(m)
            ones_col = const.tile([128, NJ, 1], BF16, tag="ones_col")
            nc.gpsimd.memset(ones_col[:], 1.0)
            # selc[h16, fc, p] = 1 iff h16 == (p//64)*8 + fc: K=16 matmul
            # broadcasting den row (hl*8+fc) onto head groups (host-built)
            selc_sb = const.tile([2 * N_CORES, 8, 128], BF16, tag="selc")
            nc.gpsimd.dma_start(selc_sb[:], selc[:])

            # ---- weights resident in SBUF ----
            wq_sb = wpool.tile([128, 8, 3 * FL], BF16, tag="wq")
            nc.sync.dma_start(wq_sb[:], wqkvT_r)
            wout_sb = wpool.tile([128, 8, D], BF16, tag="wout")
            nc.gpsimd.dma_start(wout_sb[:], woutT_r)

            # ---- internal DRAM for the per-batch staged AllToAll ----
            a2a_ins = [dram.tile([N_CORES, AROW, CH], BF16,
                                 tag=f"a2a_in{s}", name=f"a2a_in{s}")
                       for s in range(B)]
            a2a_outs = [dram.tile([N_CORES, AROW, CH], BF16,
                                  tag=f"a2a_out{s}", name=f"a2a_out{s}")
                        for s in range(B)]

            def _outproj(s):
                # stage s's 256 output tokens: needs a2a_outs[s] only
                lh = opool.tile([128, 8, CH], BF16, tag="lh",
                                name=f"lh_{s}")
                den_t = opool.tile([2 * N_CORES, CH], BF16, tag="den_t",
                                   name=f"den_t_{s}")
                for hl in range(HL):
                    r0 = hl * (DH + 1)
                    nc.sync.dma_start(
                        lh[hl * DH:(hl + 1) * DH, :, :],
                        a2a_outs[s][:, r0:r0 + DH, :].rearrange(
                            "f p t -> p f t"))
                    nc.sync.dma_start(
                        den_t[hl * 8:(hl + 1) * 8, :],
                        a2a_outs[s][:, r0 + DH, :])
                den_rf = opool.tile([2 * N_CORES, CH], F32, tag="den_rf",
                                    name=f"den_rf_{s}")
                nc.vector.reciprocal(den_rf[:], den_t[:])
                den_r = opool.tile([2 * N_CORES, CH], BF16, tag="den_r",
                                   name=f"den_r_{s}")
                nc.scalar.copy(den_r[:], den_rf[:])
                lhn = opool.tile([128, 8, CH], BF16, tag="lhn",
                                 name=f"lhn_{s}")
                for fc in range(8):
                    psn = psN.tile([128, CH], F32, tag="psn",
                                   name=f"psn_{s}_{fc}")
                    nc.tensor.matmul(psn[:], lhsT=selc_sb[:, fc, :],
                                     rhs=den_r[:],
                                     start=True, stop=True)
                    nc.vector.tensor_tensor(lhn[:, fc, :], lh[:, fc, :],
                                            psn[:], mybir.AluOpType.mult)
                for tt in range(CH // 128):
                    for db in range(D // IB):
                        pso = psA.tile([128, IB], F32, tag="ps",
                                       name=f"pso_{s}_{tt}_{db}")
                        for fc in range(8):
                            nc.tensor.matmul(
                                pso[:],
                                lhsT=lhn[:, fc, tt * 128:(tt + 1) * 128],
                                rhs=wout_sb[:, fc, db * IB:(db + 1) * IB],
                                start=(fc == 0), stop=(fc == 7),
                            )
                        osb = opool.tile([128, IB], F32, tag="osb",
                                         name=f"osb_{s}_{tt}_{db}")
                        nc.scalar.copy(osb[:], pso[:])
                        row0 = s * CH + tt * 128
                        nc.sync.dma_start(
                            out[row0:row0 + 128, db * IB:(db + 1) * IB],
                            osb[:],
                        )

            for b in range(B):
                # ---- qkv projection for batch b ----
                # q,k land transposed in qkvT[feat, tok], one 128-row slot
                # per head with the off-head rows zeroed (so the padded
                # K=128 score matmuls contract cleanly and get FWL); v is
                # staged per token-block and PE-transposed into vt = [v|1|0]
                qkvT = qkvpool.tile([128, 4, T], BF16, tag="qkvT")
                nc.vector.memset(qkvT[64:128, 0, :], 0.0)
                nc.vector.memset(qkvT[0:64, 1, :], 0.0)
                nc.vector.memset(qkvT[64:128, 2, :], 0.0)
                nc.vector.memset(qkvT[0:64, 3, :], 0.0)
                vts = []
                for hl in range(HL):
                    vt = vpool.tile([128, NJ, 128], BF16, tag="vt")
                    nc.vector.memset(vt[:, :, DH + 1:], 0.0)
                    nc.scalar.copy(vt[:, :, DH:DH + 1], ones_col[:])
                    vts.append(vt)
                for tb in range(T // IB):
                    xt = xpool.tile([128, 8, IB], BF16, tag="xt")
                    t0 = b * T + tb * IB
                    nc.sync.dma_start(xt[:], xT_r[:, :, t0:t0 + IB])
                    for ft in range(3):
                        ps = psA.tile([128, IB], F32, tag="ps")
                        for dc in range(8):
                            nc.tensor.matmul(
                                ps[:],
                                lhsT=wq_sb[:, dc, ft * 128:(ft + 1) * 128],
                                rhs=xt[:, dc, :],
                                start=(dc == 0), stop=(dc == 7),
                            )
                        if ft < 2:
                            sl = tb * IB
                            nc.vector.tensor_copy(
                                qkvT[0:64, 2 * ft, sl:sl + IB], ps[0:64, :])
                            nc.vector.tensor_copy(
                                qkvT[64:128, 2 * ft + 1, sl:sl + IB],
                                ps[64:128, :])
                        else:
                            vst = xpool.tile([128, IB], BF16, tag="vst")
                            nc.vector.tensor_copy(vst[:], ps[:])
                            for hl in range(HL):
                                for q4 in range(4):
                                    jt = tb * 4 + q4
                                    pst = psT.tile([128, 128], BF16,
                                                   tag="pst")
                                    nc.tensor.transpose(
                                        pst[:, :DH],
                                        vst[hl * DH:(hl + 1) * DH,
                                            q4 * 128:(q4 + 1) * 128],
                                        identity[hl * DH:(hl + 1) * DH,
                                                 hl * DH:(hl + 1) * DH],
                                    )
                                    nc.scalar.copy(vts[hl][:, jt, :DH],
                                                   pst[:, :DH])

                # ---- causal attention per local head ----
                for hl in range(HL):
                    vt = vts[hl]
                    q_sl = qkvT[:, hl, :]
                    k_sl = qkvT[:, 2 + hl, :]
                    for ib in range(T // IB):
                        nj = 4 * (ib + 1)
                        # phase S: score matmuls back-to-back; exp+mask
                        # drain behind them on ACT/DVE into bf16 p tiles
                        ps_list = []
                        for jt in range(nj):
                            pss = psA.tile([128, IB], F32, tag="ps")
                            # transposed scores: [j(128), i(IB)]
                            nc.tensor.matmul(
                                pss[:],
                                lhsT=k_sl[:, jt * 128:(jt + 1) * 128],
                                rhs=q_sl[:, ib * IB:(ib + 1) * IB],
                                start=True, stop=True,
                            )
                            p = ppool.tile([128, IB], BF16, tag="p")
                            nc.scalar.activation(
                                p[:], pss[:],
                                mybir.ActivationFunctionType.Exp,
                                scale=SCALE,
                            )
                            if jt >= ib * 4:  # block-diagonal: apply mask
                                nc.vector.tensor_tensor(
                                    p[:], p[:], masks[jt - ib * 4][:],
                                    mybir.AluOpType.mult,
                                )
                            ps_list.append(p)
                        # phase PV: back-to-back accumulation into one bank;
                        # psy row DH is the softmax denominator
                        psy = psY.tile([128, IB], F32, tag="psy")
                        for jt in range(nj):
                            nc.tensor.matmul(
                                psy[:], lhsT=vt[:, jt, :], rhs=ps_list[jt][:],
                                start=(jt == 0), stop=(jt == nj - 1),
                            )
                        # ship unnormalized y + den row, split across the
                        # two destination ranks of this i-block
                        yd = ydpool.tile([DH + 1, IB], BF16, tag="yd")
                        nc.vector.tensor_copy(yd[:], psy[:DH + 1, :])
                        r0 = hl * (DH + 1)
                        for c in range(2):
                            nc.gpsimd.dma_start(
                                a2a_ins[b][2 * ib + c, r0:r0 + DH + 1, :],
                                yd[:, c * CH:(c + 1) * CH],
                            )

                # stage collective fires as soon as batch b's y landed
                nc.gpsimd.collective_compute(
                    "AllToAll", mybir.AluOpType.bypass,
                    replica_groups=[list(range(N_CORES))],
                    ins=[a2a_ins[b][:]], outs=[a2a_outs[b][:]],
                )
                if b > 0:
                    _outproj(b - 1)
            _outproj(B - 1)

    nc.finalize()
    return nc


_NC_CACHE = {}


def _get_nc():
    if "nc" not in _NC_CACHE:
        _NC_CACHE["nc"] = _build()
    return _NC_CACHE["nc"]


def kernel(x, w_qkv, w_out):
    x = np.asarray(x, np.float32).reshape(BT, D)
    w_qkv = np.asarray(w_qkv, np.float32)
    w_out = np.asarray(w_out, np.float32)

    xT = np.ascontiguousarray(x.T).astype(ml_dtypes.bfloat16)
    woutT = np.ascontiguousarray(w_out.T).astype(ml_dtypes.bfloat16)

    selc = np.zeros((2 * N_CORES, 8, 128), ml_dtypes.bfloat16)
    for h16 in range(2 * N_CORES):
        hl, fc = divmod(h16, 8)
        selc[h16, fc, hl * 64:(hl + 1) * 64] = 1.0

    in_maps = []
    for c in range(N_CORES):
        rows = []
        for t in range(3):
            for hl in range(HL):
                h = HL * c + hl
                rows.append(w_qkv[t * H * DH + h * DH:
                                  t * H * DH + (h + 1) * DH])
        wq_c = np.concatenate(rows, axis=0)  # [384, D]
        in_maps.append({
            "xT": xT,
            "wqkvT": np.ascontiguousarray(wq_c.T).astype(ml_dtypes.bfloat16),
            "woutT": woutT,
            "selc": selc,
        })

    nc = _get_nc()
    do_trace = bool(os.environ.get("ATTN_TRACE"))
    if do_trace:
        _install_profile_hook()
    res = run_bass_kernel_spmd(nc, in_maps, list(range(N_CORES)),
                               trace=do_trace)
    if res.exec_time_ns is not None:
        print(f"HW exec time: {res.exec_time_ns} ns")
        _NC_CACHE["exec_time_ns"] = res.exec_time_ns
        _NC_CACHE["trace"] = res.instructions_and_trace
    # rank r's out rows are 4 x 256-token chunks, one per batch, covering
    # within-batch token chunk r: global token b*T + r*256 + t
    full = np.empty((B, N_CORES, CH, D), np.float32)
    for c in range(N_CORES):
        full[:, c] = res.results[c]["out"].reshape(B, CH, D)
    return full.reshape(B, T, D)


# revision 27
# speedup vs baseline: 1.0022x; 1.0022x over previous
"""Distributed causal self-attention on 8 TRN2 NeuronCores.

Strategy (tensor parallel on heads + per-batch staged AllToAll):
  - Each core owns 2 of the 16 heads: qkv projection for its heads (full
    batch/seq), causal attention in a transposed-score layout (scores
    [key, query] so softmax denominators come free from an appended
    ones-row on V; no row-max subtraction needed at these magnitudes).
  - Unnormalized y plus the per-(head,token) softmax denominator are
    resharded head-split -> token-split with one AllToAll per batch, so
    each collective fires as soon as its batch finishes and overlaps the
    next batch's compute. Normalization happens on the consumer side.
  - Each core computes the output projection for its token chunks.
  - Host gather is pure concatenation/reordering along tokens.

All TensorEngine-facing tensors keep the contraction dim on partitions
and use bf16 with full 128-row stationaries (q/k zero-padded per head)
so every matmul gets the fast-weight-load path. PSUM accumulation stays
fp32.
"""

import os
import sys

sys.path.insert(0, "/opt/trn_rl_repo")

import ml_dtypes
import numpy as np

import concourse.bass as bass
import concourse.mybir as mybir
import concourse.tile as tile
from concourse.tile import add_dep_helper
from concourse import bacc
from concourse.bass_utils import run_bass_kernel_spmd
from concourse.masks import make_identity


def _install_profile_hook():
    """The RL container's antenv stub lacks axon_hooks, so bass_utils can't
    reach the NTFF profiler. Recreate the tiny set/get module and wire it to
    trn_boot's ctypes hook against libaxon_pjrt.so."""
    import types

    if "antenv.axon_hooks" in sys.modules:
        return
    try:
        import antenv
        from trn_agent_boot.trn_boot import _ntff_profile_via_ctypes

        mod = types.ModuleType("antenv.axon_hooks")
        mod._hook = None

        def set_axon_ntff_profile_hook(h):
            mod._hook = h

        def get_axon_ntff_profile_hook():
            return mod._hook

        mod.set_axon_ntff_profile_hook = set_axon_ntff_profile_hook
        mod.get_axon_ntff_profile_hook = get_axon_ntff_profile_hook
        sys.modules["antenv.axon_hooks"] = mod
        antenv.axon_hooks = mod
        hook = _ntff_profile_via_ctypes("/opt/axon/libaxon_pjrt.so")
        if hook is not None:
            mod._hook = hook
    except Exception as e:  # profiling is best-effort; execution must work
        print(f"profile hook install failed: {e}", file=sys.stderr)


B, T, D, H, DH = 4, 2048, 1024, 16, 64
BT = B * T              # 8192 tokens
N_CORES = 8
HL = H // N_CORES       # 2 heads per core
FL = HL * DH            # 128 local features
TSLICE = BT // N_CORES  # 1024 output tokens per core
CH = TSLICE // B        # 256-token ownership chunk per (rank, batch)
SCALE = DH ** -0.5
F32 = mybir.dt.float32
BF16 = mybir.dt.bfloat16

IB = 512       # query block (free dim of transposed score matmuls)
NJ = T // 128  # 16 key tiles per (b, h)
AROW = FL + HL  # 130 rows per a2a block: 128 y rows + 2 den rows


def _build():
    nc = bacc.Bacc("TRN2", target_bir_lowering=False, debug=False,
                   num_devices=N_CORES)

    xT = nc.dram_tensor("xT", [D, BT], BF16, kind="ExternalInput")
    wqkvT = nc.dram_tensor("wqkvT", [D, 3 * FL], BF16, kind="ExternalInput")
    woutT = nc.dram_tensor("woutT", [D, D], BF16, kind="ExternalInput")
    selc = nc.dram_tensor("selc", [2 * N_CORES, 8, 128], BF16,
                          kind="ExternalInput")
    out = nc.dram_tensor("out", [TSLICE, D], F32, kind="ExternalOutput")

    xT_r = xT[:].rearrange("(o p) t -> p o t", p=128)        # [128, 8, BT]
    wqkvT_r = wqkvT[:].rearrange("(o p) f -> p o f", p=128)  # [128, 8, 384]
    woutT_r = woutT[:].rearrange("(o p) d -> p o d", p=128)  # [128, 8, 1024]

    with tile.TileContext(nc) as tc:
        from contextlib import ExitStack

        with ExitStack() as ctx:
            const = ctx.enter_context(tc.tile_pool(name="const", bufs=1))
            wpool = ctx.enter_context(tc.tile_pool(name="wpool", bufs=1))
            xpool = ctx.enter_context(tc.tile_pool(name="xpool", bufs=3))
            qkvpool = ctx.enter_context(tc.tile_pool(name="qkvpool", bufs=3))
            vpool = ctx.enter_context(tc.tile_pool(name="vpool", bufs=6))
            ppool = ctx.enter_context(tc.tile_pool(name="ppool", bufs=24))
            ydpool = ctx.enter_context(tc.tile_pool(name="ydpool", bufs=4))
            opool = ctx.enter_context(tc.tile_pool(name="opool", bufs=2))
            psA = ctx.enter_context(
                tc.tile_pool(name="psA", bufs=3, space="PSUM"))
            psY = ctx.enter_context(
                tc.tile_pool(name="psY", bufs=2, space="PSUM"))
            psT = ctx.enter_context(
                tc.tile_pool(name="psT", bufs=2, space="PSUM"))
            psN = ctx.enter_context(
                tc.tile_pool(name="psN", bufs=1, space="PSUM"))
            dram = ctx.enter_context(
                tc.tile_pool(name="dram", bufs=1, space="DRAM"))

            # ---- constants ----
            identity = const.tile([128, 128], BF16, tag="identity")
            make_identity(nc, identity[:])
            # masks[oi][p, f] = 1.0 where f - p - oi*128 >= 0 else 0
            # (keep key j0+p for query i0+f iff j <= i; offset oi*128 = j0-i0)
            masks = []
            for oi in range(4):
                m = const.tile([128, IB], BF16, tag=f"mask{oi}")
                nc.gpsimd.memset(m[:], 1.0)
                nc.gpsimd.affine_select(
                    out=m[:], in_=m[:],
                    compare_op=mybir.AluOpType.is_ge,
                    fill=0.0, base=-(oi * 128),
                    pattern=[[1, IB]], channel_multiplier=-1,
                )
                masks.append(m)
            ones_col = const.tile([128, NJ, 1], BF16, tag="ones_col")
            nc.gpsimd.memset(ones_col[:], 1.0)
            # selc[h16, fc, p] = 1 iff h16 == (p//64)*8 + fc: K=16 matmul
            # broadcasting den row (hl*8+fc) onto head groups (host-built)
            selc_sb = const.tile([2 * N_CORES, 8, 128], BF16, tag="selc")
            nc.gpsimd.dma_start(selc_sb[:], selc[:])

            # ---- weights resident in SBUF ----
            wq_sb = wpool.tile([128, 8, 3 * FL], BF16, tag="wq")
            nc.sync.dma_start(wq_sb[:], wqkvT_r)
            wout_sb = wpool.tile([128, 8, D], BF16, tag="wout")
            nc.gpsimd.dma_start(wout_sb[:], woutT_r)

            # ---- internal DRAM for the per-batch staged AllToAll ----
            a2a_ins = [dram.tile([N_CORES, AROW, CH], BF16,
                                 tag=f"a2a_in{s}", name=f"a2a_in{s}")
                       for s in range(B)]
            a2a_outs = [dram.tile([N_CORES, AROW, CH], BF16,
                                  tag=f"a2a_out{s}", name=f"a2a_out{s}")
                        for s in range(B)]

            anchors = [None] * B

            def _anchor(inst, s):
                # keep outproj(s) out of the engine streams until batch s+1's
                # attention is mostly issued — the scheduler's cost model
                # underestimates collective latency and would hoist these
                # ahead, head-of-line-blocking the PE on the AllToAll
                if s + 1 < B and anchors[s + 1] is not None:
                    add_dep_helper(inst.ins, anchors[s + 1], sync=False,
                                   reason="outproj stays behind next batch")
                return inst

            def _outproj(s):
                # stage s's 256 output tokens: needs a2a_outs[s] only
                lh = opool.tile([128, 8, CH], BF16, tag="lh",
                                name=f"lh_{s}")
                den_t = opool.tile([2 * N_CORES, CH], BF16, tag="den_t",
                                   name=f"den_t_{s}")
                for hl in range(HL):
                    r0 = hl * (DH + 1)
                    _anchor(nc.sync.dma_start(
                        lh[hl * DH:(hl + 1) * DH, :, :],
                        a2a_outs[s][:, r0:r0 + DH, :].rearrange(
                            "f p t -> p f t")), s)
                    _anchor(nc.sync.dma_start(
                        den_t[hl * 8:(hl + 1) * 8, :],
                        a2a_outs[s][:, r0 + DH, :]), s)
                den_rf = opool.tile([2 * N_CORES, CH], F32, tag="den_rf",
                                    name=f"den_rf_{s}")
                _anchor(nc.vector.reciprocal(den_rf[:], den_t[:]), s)
                den_r = opool.tile([2 * N_CORES, CH], BF16, tag="den_r",
                                   name=f"den_r_{s}")
                nc.scalar.copy(den_r[:], den_rf[:])
                lhn = opool.tile([128, 8, CH], BF16, tag="lhn",
                                 name=f"lhn_{s}")
                for fc in range(8):
                    psn = psN.tile([128, CH], F32, tag="psn",
                                   name=f"psn_{s}_{fc}")
                    _anchor(nc.tensor.matmul(psn[:], lhsT=selc_sb[:, fc, :],
                                             rhs=den_r[:],
                                             start=True, stop=True), s)
                    nc.vector.tensor_tensor(lhn[:, fc, :], lh[:, fc, :],
                                            psn[:], mybir.AluOpType.mult)
                for tt in range(CH // 128):
                    for db in range(D // IB):
                        pso = psA.tile([128, IB], F32, tag="ps",
                                       name=f"pso_{s}_{tt}_{db}")
                        for fc in range(8):
                            nc.tensor.matmul(
                                pso[:],
                                lhsT=lhn[:, fc, tt * 128:(tt + 1) * 128],
                                rhs=wout_sb[:, fc, db * IB:(db + 1) * IB],
                                start=(fc == 0), stop=(fc == 7),
                            )
                        osb = opool.tile([128, IB], F32, tag="osb",
                                         name=f"osb_{s}_{tt}_{db}")
                        nc.scalar.copy(osb[:], pso[:])
                        row0 = s * CH + tt * 128
                        nc.sync.dma_start(
                            out[row0:row0 + 128, db * IB:(db + 1) * IB],
                            osb[:],
                        )

            for b in range(B):
                # ---- qkv projection for batch b ----
                # q,k land transposed in qkvT[feat, tok], one 128-row slot
                # per head with the off-head rows zeroed (so the padded
                # K=128 score matmuls contract cleanly and get FWL); v is
                # staged per token-block and PE-transposed into vt = [v|1|0]
                qkvT = qkvpool.tile([128, 4, T], BF16, tag="qkvT")
                nc.vector.memset(qkvT[64:128, 0, :], 0.0)
                nc.vector.memset(qkvT[0:64, 1, :], 0.0)
                nc.vector.memset(qkvT[64:128, 2, :], 0.0)
                nc.vector.memset(qkvT[0:64, 3, :], 0.0)
                vts = []
                for hl in range(HL):
                    vt = vpool.tile([128, NJ, 128], BF16, tag="vt")
                    nc.vector.memset(vt[:, :, DH + 1:], 0.0)
                    nc.scalar.copy(vt[:, :, DH:DH + 1], ones_col[:])
                    vts.append(vt)
                for tb in range(T // IB):
                    xt = xpool.tile([128, 8, IB], BF16, tag="xt")
                    t0 = b * T + tb * IB
                    nc.sync.dma_start(xt[:], xT_r[:, :, t0:t0 + IB])
                    for ft in range(3):
                        ps = psA.tile([128, IB], F32, tag="ps")
                        for dc in range(8):
                            nc.tensor.matmul(
                                ps[:],
                                lhsT=wq_sb[:, dc, ft * 128:(ft + 1) * 128],
                                rhs=xt[:, dc, :],
                                start=(dc == 0), stop=(dc == 7),
                            )
                        if ft < 2:
                            sl = tb * IB
                            nc.vector.tensor_copy(
                                qkvT[0:64, 2 * ft, sl:sl + IB], ps[0:64, :])
                            nc.vector.tensor_copy(
                                qkvT[64:128, 2 * ft + 1, sl:sl + IB],
                                ps[64:128, :])
                        else:
                            vst = xpool.tile([128, IB], BF16, tag="vst")
                            nc.vector.tensor_copy(vst[:], ps[:])
                            for hl in range(HL):
                                for q4 in range(4):
                                    jt = tb * 4 + q4
                                    pst = psT.tile([128, 128], BF16,
                                                   tag="pst")
                                    nc.tensor.transpose(
                                        pst[:, :DH],
                                        vst[hl * DH:(hl + 1) * DH,
                                            q4 * 128:(q4 + 1) * 128],
                                        identity[hl * DH:(hl + 1) * DH,
                                                 hl * DH:(hl + 1) * DH],
                                    )
                                    nc.scalar.copy(vts[hl][:, jt, :DH],
                                                   pst[:, :DH])

                # ---- causal attention per local head ----
                for hl in range(HL):
                    vt = vts[hl]
                    q_sl = qkvT[:, hl, :]
                    k_sl = qkvT[:, 2 + hl, :]
                    for ib in range(T // IB):
                        nj = 4 * (ib + 1)
                        # phase S: score matmuls back-to-back; exp+mask
                        # drain behind them on ACT/DVE into bf16 p tiles
                        ps_list = []
                        for jt in range(nj):
                            pss = psA.tile([128, IB], F32, tag="ps")
                            # transposed scores: [j(128), i(IB)]
                            anchors[b] = nc.tensor.matmul(
                                pss[:],
                                lhsT=k_sl[:, jt * 128:(jt + 1) * 128],
                                rhs=q_sl[:, ib * IB:(ib + 1) * IB],
                                start=True, stop=True,
                            ).ins
                            p = ppool.tile([128, IB], BF16, tag="p")
                            nc.scalar.activation(
                                p[:], pss[:],
                                mybir.ActivationFunctionType.Exp,
                                scale=SCALE,
                            )
                            if jt >= ib * 4:  # block-diagonal: apply mask
                                nc.vector.tensor_tensor(
                                    p[:], p[:], masks[jt - ib * 4][:],
                                    mybir.AluOpType.mult,
                                )
                            ps_list.append(p)
                        # phase PV: back-to-back accumulation into one bank;
                        # psy row DH is the softmax denominator
                        psy = psY.tile([128, IB], F32, tag="psy")
                        for jt in range(nj):
                            nc.tensor.matmul(
                                psy[:], lhsT=vt[:, jt, :], rhs=ps_list[jt][:],
                                start=(jt == 0), stop=(jt == nj - 1),
                            )
                        # ship unnormalized y + den row, split across the
                        # two destination ranks of this i-block
                        yd = ydpool.tile([DH + 1, IB], BF16, tag="yd")
                        nc.vector.tensor_copy(yd[:], psy[:DH + 1, :])
                        r0 = hl * (DH + 1)
                        for c in range(2):
                            nc.gpsimd.dma_start(
                                a2a_ins[b][2 * ib + c, r0:r0 + DH + 1, :],
                                yd[:, c * CH:(c + 1) * CH],
                            )

                # stage collective fires as soon as batch b's y landed
                nc.gpsimd.collective_compute(
                    "AllToAll", mybir.AluOpType.bypass,
                    replica_groups=[list(range(N_CORES))],
                    ins=[a2a_ins[b][:]], outs=[a2a_outs[b][:]],
                )
                if b > 0:
                    _outproj(b - 1)
            _outproj(B - 1)

    nc.finalize()
    return nc


_NC_CACHE = {}


def _get_nc():
    if "nc" not in _NC_CACHE:
        _NC_CACHE["nc"] = _build()
    return _NC_CACHE["nc"]


def kernel(x, w_qkv, w_out):
    x = np.asarray(x, np.float32).reshape(BT, D)
    w_qkv = np.asarray(w_qkv, np.float32)
    w_out = np.asarray(w_out, np.float32)

    xT = np.ascontiguousarray(x.T).astype(ml_dtypes.bfloat16)
    woutT = np.ascontiguousarray(w_out.T).astype(ml_dtypes.bfloat16)

    selc = np.zeros((2 * N_CORES, 8, 128), ml_dtypes.bfloat16)
    for h16 in range(2 * N_CORES):
        hl, fc = divmod(h16, 8)
        selc[h16, fc, hl * 64:(hl + 1) * 64] = 1.0

    in_maps = []
    for c in range(N_CORES):
        rows = []
        for t in range(3):
            for hl in range(HL):
                h = HL * c + hl
                rows.append(w_qkv[t * H * DH + h * DH:
                                  t * H * DH + (h + 1) * DH])
        wq_c = np.concatenate(rows, axis=0)  # [384, D]
        in_maps.append({
            "xT": xT,
            "wqkvT": np.ascontiguousarray(wq_c.T).astype(ml_dtypes.bfloat16),
            "woutT": woutT,
            "selc": selc,
        })

    nc = _get_nc()
    do_trace = bool(os.environ.get("ATTN_TRACE"))
    if do_trace:
        _install_profile_hook()
    res = run_bass_kernel_spmd(nc, in_maps, list(range(N_CORES)),
                               trace=do_trace)
    if res.exec_time_ns is not None:
        print(f"HW exec time: {res.exec_time_ns} ns")
        _NC_CACHE["exec_time_ns"] = res.exec_time_ns
        _NC_CACHE["trace"] = res.instructions_and_trace
    # rank r's out rows are 4 x 256-token chunks, one per batch, covering
    # within-batch token chunk r: global token b*T + r*256 + t
    full = np.empty((B, N_CORES, CH, D), np.float32)
    for c in range(N_CORES):
        full[:, c] = res.results[c]["out"].reshape(B, CH, D)
    return full.reshape(B, T, D)


# revision 28
# speedup vs baseline: 1.0319x; 1.0296x over previous
"""Distributed causal self-attention on 8 TRN2 NeuronCores.

Strategy (tensor parallel on heads + per-batch staged AllToAll):
  - Each core owns 2 of the 16 heads: qkv projection for its heads (full
    batch/seq), causal attention in a transposed-score layout (scores
    [key, query] so softmax denominators come free from an appended
    ones-row on V; no row-max subtraction needed at these magnitudes).
  - Unnormalized y plus the per-(head,token) softmax denominator are
    resharded head-split -> token-split with one AllToAll per batch, so
    each collective fires as soon as its batch finishes and overlaps the
    next batch's compute. Normalization happens on the consumer side.
  - Each core computes the output projection for its token chunks.
  - Host gather is pure concatenation/reordering along tokens.

All TensorEngine-facing tensors keep the contraction dim on partitions
and use bf16 with full 128-row stationaries (q/k zero-padded per head)
so every matmul gets the fast-weight-load path. PSUM accumulation stays
fp32.
"""

import os
import sys

sys.path.insert(0, "/opt/trn_rl_repo")

import ml_dtypes
import numpy as np

import concourse.bass as bass
import concourse.mybir as mybir
import concourse.tile as tile
from concourse.tile import add_dep_helper
from concourse import bacc
from concourse.bass_utils import run_bass_kernel_spmd
from concourse.masks import make_identity


def _install_profile_hook():
    """The RL container's antenv stub lacks axon_hooks, so bass_utils can't
    reach the NTFF profiler. Recreate the tiny set/get module and wire it to
    trn_boot's ctypes hook against libaxon_pjrt.so."""
    import types

    if "antenv.axon_hooks" in sys.modules:
        return
    try:
        import antenv
        from trn_agent_boot.trn_boot import _ntff_profile_via_ctypes

        mod = types.ModuleType("antenv.axon_hooks")
        mod._hook = None

        def set_axon_ntff_profile_hook(h):
            mod._hook = h

        def get_axon_ntff_profile_hook():
            return mod._hook

        mod.set_axon_ntff_profile_hook = set_axon_ntff_profile_hook
        mod.get_axon_ntff_profile_hook = get_axon_ntff_profile_hook
        sys.modules["antenv.axon_hooks"] = mod
        antenv.axon_hooks = mod
        hook = _ntff_profile_via_ctypes("/opt/axon/libaxon_pjrt.so")
        if hook is not None:
            mod._hook = hook
    except Exception as e:  # profiling is best-effort; execution must work
        print(f"profile hook install failed: {e}", file=sys.stderr)


B, T, D, H, DH = 4, 2048, 1024, 16, 64
BT = B * T              # 8192 tokens
N_CORES = 8
HL = H // N_CORES       # 2 heads per core
FL = HL * DH            # 128 local features
TSLICE = BT // N_CORES  # 1024 output tokens per core
CH = TSLICE // B        # 256-token ownership chunk per (rank, batch)
SCALE = DH ** -0.5
F32 = mybir.dt.float32
BF16 = mybir.dt.bfloat16

IB = 512       # query block (free dim of transposed score matmuls)
NJ = T // 128  # 16 key tiles per (b, h)
AROW = FL + HL  # 130 rows per a2a block: 128 y rows + 2 den rows


def _build():
    nc = bacc.Bacc("TRN2", target_bir_lowering=False, debug=False,
                   num_devices=N_CORES)

    xT = nc.dram_tensor("xT", [D, BT], BF16, kind="ExternalInput")
    wqkvT = nc.dram_tensor("wqkvT", [D, 3 * FL], BF16, kind="ExternalInput")
    woutT = nc.dram_tensor("woutT", [D, D], BF16, kind="ExternalInput")
    selc = nc.dram_tensor("selc", [2 * N_CORES, 8, 128], BF16,
                          kind="ExternalInput")
    out = nc.dram_tensor("out", [TSLICE, D], F32, kind="ExternalOutput")

    xT_r = xT[:].rearrange("(o p) t -> p o t", p=128)        # [128, 8, BT]
    wqkvT_r = wqkvT[:].rearrange("(o p) f -> p o f", p=128)  # [128, 8, 384]
    woutT_r = woutT[:].rearrange("(o p) d -> p o d", p=128)  # [128, 8, 1024]

    with tile.TileContext(nc) as tc:
        from contextlib import ExitStack

        with ExitStack() as ctx:
            const = ctx.enter_context(tc.tile_pool(name="const", bufs=1))
            wpool = ctx.enter_context(tc.tile_pool(name="wpool", bufs=1))
            xpool = ctx.enter_context(tc.tile_pool(name="xpool", bufs=3))
            qkvpool = ctx.enter_context(tc.tile_pool(name="qkvpool", bufs=3))
            vpool = ctx.enter_context(tc.tile_pool(name="vpool", bufs=6))
            ppool = ctx.enter_context(tc.tile_pool(name="ppool", bufs=24))
            ydpool = ctx.enter_context(tc.tile_pool(name="ydpool", bufs=4))
            opool = ctx.enter_context(tc.tile_pool(name="opool", bufs=2))
            psA = ctx.enter_context(
                tc.tile_pool(name="psA", bufs=3, space="PSUM"))
            psY = ctx.enter_context(
                tc.tile_pool(name="psY", bufs=2, space="PSUM"))
            psT = ctx.enter_context(
                tc.tile_pool(name="psT", bufs=2, space="PSUM"))
            psN = ctx.enter_context(
                tc.tile_pool(name="psN", bufs=1, space="PSUM"))
            dram = ctx.enter_context(
                tc.tile_pool(name="dram", bufs=1, space="DRAM"))

            # ---- constants ----
            identity = const.tile([128, 128], BF16, tag="identity")
            make_identity(nc, identity[:])
            # masks[oi][p, f] = 1.0 where f - p - oi*128 >= 0 else 0
            # (keep key j0+p for query i0+f iff j <= i; offset oi*128 = j0-i0)
            masks = []
            for oi in range(4):
                m = const.tile([128, IB], BF16, tag=f"mask{oi}")
                nc.gpsimd.memset(m[:], 1.0)
                nc.gpsimd.affine_select(
                    out=m[:], in_=m[:],
                    compare_op=mybir.AluOpType.is_ge,
                    fill=0.0, base=-(oi * 128),
                    pattern=[[1, IB]], channel_multiplier=-1,
                )
                masks.append(m)
            ones_col = const.tile([128, NJ, 1], BF16, tag="ones_col")
            nc.gpsimd.memset(ones_col[:], 1.0)
            # selc[h16, fc, p] = 1 iff h16 == (p//64)*8 + fc: K=16 matmul
            # broadcasting den row (hl*8+fc) onto head groups (host-built)
            selc_sb = const.tile([2 * N_CORES, 8, 128], BF16, tag="selc")
            nc.gpsimd.dma_start(selc_sb[:], selc[:])

            # ---- weights resident in SBUF ----
            wq_sb = wpool.tile([128, 8, 3 * FL], BF16, tag="wq")
            nc.sync.dma_start(wq_sb[:], wqkvT_r)
            wout_sb = wpool.tile([128, 8, D], BF16, tag="wout")
            nc.gpsimd.dma_start(wout_sb[:], woutT_r)

            # ---- internal DRAM for the per-batch staged AllToAll ----
            a2a_ins = [dram.tile([N_CORES, AROW, CH], BF16,
                                 tag=f"a2a_in{s}", name=f"a2a_in{s}")
                       for s in range(B)]
            a2a_outs = [dram.tile([N_CORES, AROW, CH], BF16,
                                  tag=f"a2a_out{s}", name=f"a2a_out{s}")
                        for s in range(B)]

            anchors = [None] * B
            pv_anchors = [None] * B

            def _anchor(inst, s):
                # keep outproj(s) out of the engine streams until batch s+1's
                # attention is mostly issued — the scheduler's cost model
                # underestimates collective latency and would hoist these
                # ahead, head-of-line-blocking the PE on the AllToAll
                a = None
                if s == B - 2:
                    a = pv_anchors[B - 1]
                elif s + 1 < B:
                    a = anchors[s + 1]
                if a is not None:
                    add_dep_helper(inst.ins, a, sync=False,
                                   reason="outproj stays behind next batch")
                return inst

            def _outproj(s):
                # stage s's 256 output tokens: needs a2a_outs[s] only
                lh = opool.tile([128, 8, CH], BF16, tag="lh",
                                name=f"lh_{s}")
                den_t = opool.tile([2 * N_CORES, CH], BF16, tag="den_t",
                                   name=f"den_t_{s}")
                for hl in range(HL):
                    r0 = hl * (DH + 1)
                    _anchor(nc.sync.dma_start(
                        lh[hl * DH:(hl + 1) * DH, :, :],
                        a2a_outs[s][:, r0:r0 + DH, :].rearrange(
                            "f p t -> p f t")), s)
                    _anchor(nc.sync.dma_start(
                        den_t[hl * 8:(hl + 1) * 8, :],
                        a2a_outs[s][:, r0 + DH, :]), s)
                den_rf = opool.tile([2 * N_CORES, CH], F32, tag="den_rf",
                                    name=f"den_rf_{s}")
                _anchor(nc.vector.reciprocal(den_rf[:], den_t[:]), s)
                den_r = opool.tile([2 * N_CORES, CH], BF16, tag="den_r",
                                   name=f"den_r_{s}")
                nc.scalar.copy(den_r[:], den_rf[:])
                lhn = opool.tile([128, 8, CH], BF16, tag="lhn",
                                 name=f"lhn_{s}")
                for fc in range(8):
                    psn = psN.tile([128, CH], F32, tag="psn",
                                   name=f"psn_{s}_{fc}")
                    _anchor(nc.tensor.matmul(psn[:], lhsT=selc_sb[:, fc, :],
                                             rhs=den_r[:],
                                             start=True, stop=True), s)
                    nc.vector.tensor_tensor(lhn[:, fc, :], lh[:, fc, :],
                                            psn[:], mybir.AluOpType.mult)
                for tt in range(CH // 128):
                    for db in range(D // IB):
                        pso = psA.tile([128, IB], F32, tag="ps",
                                       name=f"pso_{s}_{tt}_{db}")
                        for fc in range(8):
                            nc.tensor.matmul(
                                pso[:],
                                lhsT=lhn[:, fc, tt * 128:(tt + 1) * 128],
                                rhs=wout_sb[:, fc, db * IB:(db + 1) * IB],
                                start=(fc == 0), stop=(fc == 7),
                            )
                        osb = opool.tile([128, IB], F32, tag="osb",
                                         name=f"osb_{s}_{tt}_{db}")
                        nc.scalar.copy(osb[:], pso[:])
                        row0 = s * CH + tt * 128
                        nc.sync.dma_start(
                            out[row0:row0 + 128, db * IB:(db + 1) * IB],
                            osb[:],
                        )

            for b in range(B):
                # ---- qkv projection for batch b ----
                # q,k land transposed in qkvT[feat, tok], one 128-row slot
                # per head with the off-head rows zeroed (so the padded
                # K=128 score matmuls contract cleanly and get FWL); v is
                # staged per token-block and PE-transposed into vt = [v|1|0]
                qkvT = qkvpool.tile([128, 4, T], BF16, tag="qkvT")
                nc.vector.memset(qkvT[64:128, 0, :], 0.0)
                nc.vector.memset(qkvT[0:64, 1, :], 0.0)
                nc.vector.memset(qkvT[64:128, 2, :], 0.0)
                nc.vector.memset(qkvT[0:64, 3, :], 0.0)
                vts = []
                for hl in range(HL):
                    vt = vpool.tile([128, NJ, 128], BF16, tag="vt")
                    nc.vector.memset(vt[:, :, DH + 1:], 0.0)
                    nc.scalar.copy(vt[:, :, DH:DH + 1], ones_col[:])
                    vts.append(vt)
                for tb in range(T // IB):
                    xt = xpool.tile([128, 8, IB], BF16, tag="xt")
                    t0 = b * T + tb * IB
                    nc.sync.dma_start(xt[:], xT_r[:, :, t0:t0 + IB])
                    for ft in range(3):
                        ps = psA.tile([128, IB], F32, tag="ps")
                        for dc in range(8):
                            nc.tensor.matmul(
                                ps[:],
                                lhsT=wq_sb[:, dc, ft * 128:(ft + 1) * 128],
                                rhs=xt[:, dc, :],
                                start=(dc == 0), stop=(dc == 7),
                            )
                        if ft < 2:
                            sl = tb * IB
                            nc.vector.tensor_copy(
                                qkvT[0:64, 2 * ft, sl:sl + IB], ps[0:64, :])
                            nc.vector.tensor_copy(
                                qkvT[64:128, 2 * ft + 1, sl:sl + IB],
                                ps[64:128, :])
                        else:
                            vst = xpool.tile([128, IB], BF16, tag="vst")
                            nc.vector.tensor_copy(vst[:], ps[:])
                            for hl in range(HL):
                                for q4 in range(4):
                                    jt = tb * 4 + q4
                                    pst = psT.tile([128, 128], BF16,
                                                   tag="pst")
                                    nc.tensor.transpose(
                                        pst[:, :DH],
                                        vst[hl * DH:(hl + 1) * DH,
                                            q4 * 128:(q4 + 1) * 128],
                                        identity[hl * DH:(hl + 1) * DH,
                                                 hl * DH:(hl + 1) * DH],
                                    )
                                    nc.scalar.copy(vts[hl][:, jt, :DH],
                                                   pst[:, :DH])

                # ---- causal attention per local head ----
                for hl in range(HL):
                    vt = vts[hl]
                    q_sl = qkvT[:, hl, :]
                    k_sl = qkvT[:, 2 + hl, :]
                    for ib in range(T // IB):
                        nj = 4 * (ib + 1)
                        # phase S: score matmuls back-to-back; exp+mask
                        # drain behind them on ACT/DVE into bf16 p tiles
                        ps_list = []
                        for jt in range(nj):
                            pss = psA.tile([128, IB], F32, tag="ps")
                            # transposed scores: [j(128), i(IB)]
                            mm = nc.tensor.matmul(
                                pss[:],
                                lhsT=k_sl[:, jt * 128:(jt + 1) * 128],
                                rhs=q_sl[:, ib * IB:(ib + 1) * IB],
                                start=True, stop=True,
                            )
                            if hl == 1 and ib == 1 and jt == 0:
                                anchors[b] = mm.ins
                            p = ppool.tile([128, IB], BF16, tag="p")
                            nc.scalar.activation(
                                p[:], pss[:],
                                mybir.ActivationFunctionType.Exp,
                                scale=SCALE,
                            )
                            if jt >= ib * 4:  # block-diagonal: apply mask
                                nc.vector.tensor_tensor(
                                    p[:], p[:], masks[jt - ib * 4][:],
                                    mybir.AluOpType.mult,
                                )
                            ps_list.append(p)
                        # phase PV: back-to-back accumulation into one bank;
                        # psy row DH is the softmax denominator
                        psy = psY.tile([128, IB], F32, tag="psy")
                        for jt in range(nj):
                            mm = nc.tensor.matmul(
                                psy[:], lhsT=vt[:, jt, :], rhs=ps_list[jt][:],
                                start=(jt == 0), stop=(jt == nj - 1),
                            )
                        pv_anchors[b] = mm.ins
                        # ship unnormalized y + den row, split across the
                        # two destination ranks of this i-block
                        yd = ydpool.tile([DH + 1, IB], BF16, tag="yd")
                        nc.vector.tensor_copy(yd[:], psy[:DH + 1, :])
                        r0 = hl * (DH + 1)
                        for c in range(2):
                            nc.gpsimd.dma_start(
                                a2a_ins[b][2 * ib + c, r0:r0 + DH + 1, :],
                                yd[:, c * CH:(c + 1) * CH],
                            )

                # stage collective fires as soon as batch b's y landed
                nc.gpsimd.collective_compute(
                    "AllToAll", mybir.AluOpType.bypass,
                    replica_groups=[list(range(N_CORES))],
                    ins=[a2a_ins[b][:]], outs=[a2a_outs[b][:]],
                )
                if b > 0:
                    _outproj(b - 1)
            _outproj(B - 1)

    nc.finalize()
    return nc


_NC_CACHE = {}


def _get_nc():
    if "nc" not in _NC_CACHE:
        _NC_CACHE["nc"] = _build()
    return _NC_CACHE["nc"]


def kernel(x, w_qkv, w_out):
    x = np.asarray(x, np.float32).reshape(BT, D)
    w_qkv = np.asarray(w_qkv, np.float32)
    w_out = np.asarray(w_out, np.float32)

    xT = np.ascontiguousarray(x.T).astype(ml_dtypes.bfloat16)
    woutT = np.ascontiguousarray(w_out.T).astype(ml_dtypes.bfloat16)

    selc = np.zeros((2 * N_CORES, 8, 128), ml_dtypes.bfloat16)
    for h16 in range(2 * N_CORES):
        hl, fc = divmod(h16, 8)
        selc[h16, fc, hl * 64:(hl + 1) * 64] = 1.0

    in_maps = []
    for c in range(N_CORES):
        rows = []
        for t in range(3):
            for hl in range(HL):
                h = HL * c + hl
                rows.append(w_qkv[t * H * DH + h * DH:
                                  t * H * DH + (h + 1) * DH])
        wq_c = np.concatenate(rows, axis=0)  # [384, D]
        in_maps.append({
            "xT": xT,
            "wqkvT": np.ascontiguousarray(wq_c.T).astype(ml_dtypes.bfloat16),
            "woutT": woutT,
            "selc": selc,
        })

    nc = _get_nc()
    do_trace = bool(os.environ.get("ATTN_TRACE"))
    if do_trace:
        _install_profile_hook()
    res = run_bass_kernel_spmd(nc, in_maps, list(range(N_CORES)),
                               trace=do_trace)
    if res.exec_time_ns is not None:
        print(f"HW exec time: {res.exec_time_ns} ns")
        _NC_CACHE["exec_time_ns"] = res.exec_time_ns
        _NC_CACHE["trace"] = res.instructions_and_trace
    # rank r's out rows are 4 x 256-token chunks, one per batch, covering
    # within-batch token chunk r: global token b*T + r*256 + t
    full = np.empty((B, N_CORES, CH, D), np.float32)
    for c in range(N_CORES):
        full[:, c] = res.results[c]["out"].reshape(B, CH, D)
    return full.reshape(B, T, D)
